# revision 6
# baseline (speedup 1.0000x reference)
"""Trainium2 Bass kernel for nn_LossLayer (distance loss_fn).

reference semantics:
    dist[b, r] = || w[b] - r_emb[r] ||_2         (B=4096, R=64, D=512)
    pred = one_hot(argmax_r dist)                [B, R] f32
    y = argmax_r label
    plus = dist[b, y]
    minus = dist[b, top1] if top1 != y else dist[b, top2]
    loss = mean(1 + plus - minus)                scalar f32

Strategy (data-parallel over batch on 8 cores, r_embedding replicated):
    s[b, r] = ||r_r||^2 - 2 w_b . r_r   (= dist^2 - ||w_b||^2, same per-row order)
    All argmax / top-2 / gather selections are done on s (exact f32 values from
    PSUM; sqrt is monotonic so ordering matches dist). sqrt is needed only for
    the 2 selected values per row (plus/minus); computed with the rsqrt
    bit-trick + 3 Newton iterations on the vector engine (f32-accurate,
    avoiding the low-precision ACT sqrt table).
    Each core emits its pred shard [512, 64] and a partial sum of
    (plus - minus) over its rows; host combines: loss = 1 + total/B.
"""

import sys

import numpy as np

try:
    import concourse.bass as bass
except ImportError:  # pragma: no cover
    sys.path.insert(0, "/opt/trn_rl_repo")
    import concourse.bass as bass

import json

import concourse.mybir as mybir
from concourse import bass2jax, bass_utils, masks
from concourse.bass_utils import run_bass_kernel_spmd
from concourse.tile import TileContext

# ---------------------------------------------------------------------------
# This container's walrus build rejects (a) the EVENT_SEMAPHORE_RANGE_CLEAR
# encoding Tile emits at kernel end ("ISA wrong length") and (b) instructions
# carrying more than one sync wait ("Too many sync wait commands").  Rewrite
# the BIR JSON just before walrus: drop the range-clear, and hoist extra
# waits onto standalone single-wait EventSemaphore instructions inserted
# immediately before (same engine => per-engine order preserved).
# ---------------------------------------------------------------------------


def _rewrite_bir(j):
    for fn in j.get("functions", []):
        for blk in fn.get("blocks", []):
            out = []
            for inst in blk.get("instructions", []):
                if inst.get("isa_opcode") == 176:  # EVSEM RANGE_CLEAR
                    continue
                si = inst.get("sync_info")
                waits = (si or {}).get("on_wait") or []
                if len(waits) > 1:
                    for idx, wv in enumerate(waits[:-1]):
                        out.append({
                            "debug": inst.get("debug", 0),
                            "engine": inst["engine"],
                            "ins": [],
                            "outs": [],
                            "name": f"{inst['name']}-wsplit{idx}",
                            "opcode": "EventSemaphore",
                            "sync_info": {"on_update": [], "on_wait": [wv]},
                        })
                    si["on_wait"] = [waits[-1]]
                out.append(inst)
            blk["instructions"] = out
    return j


_ORIG_COMPILE = bass_utils.compile_bir_kernel


def _patched_compile_bir_kernel(bir_json, *args, **kwargs):
    if isinstance(bir_json, (bytes, bytearray)):
        j = json.loads(bir_json)
    else:
        j = json.loads(bir_json)
    bir_json = json.dumps(_rewrite_bir(j)).encode()
    return _ORIG_COMPILE(bir_json, *args, **kwargs)


if getattr(bass_utils.compile_bir_kernel, "__name__", "") != "_patched_compile_bir_kernel":
    bass_utils.compile_bir_kernel = _patched_compile_bir_kernel
    bass2jax.compile_bir_kernel = _patched_compile_bir_kernel

F32 = mybir.dt.float32
U32 = mybir.dt.uint32
OP = mybir.AluOpType
AX = mybir.AxisListType

B, R, D = 4096, 64, 512
N_CORES = 8
BS = B // N_CORES          # 512 rows per core
P = 128                    # partitions
NB = BS // P               # 4 batch tiles per core
ND = D // P                # 4 contraction chunks

RSQRT_MAGIC = 0x5F3759DF


def build_bass():
    nc = bass.Bass()
    w = nc.dram_tensor("w", [BS, D], F32, kind="ExternalInput")
    r = nc.dram_tensor("r", [R, D], F32, kind="ExternalInput")
    lab = nc.dram_tensor("lab", [BS, R], F32, kind="ExternalInput")
    pred = nc.dram_tensor("pred", [BS, R], F32, kind="ExternalOutput")
    lsum = nc.dram_tensor("lsum", [1, 1], F32, kind="ExternalOutput")

    with (
        TileContext(nc) as tc,
        tc.tile_pool(name="const", bufs=1) as cons,
        tc.tile_pool(name="rprep", bufs=1) as rp,
        tc.tile_pool(name="wload", bufs=2) as wload,
        tc.tile_pool(name="wt", bufs=2) as wtp,
        tc.tile_pool(name="small", bufs=2) as sm,
        tc.tile_pool(name="ps_t", bufs=2, space="PSUM") as ps_t,
        tc.tile_pool(name="ps_d", bufs=2, space="PSUM") as ps_d,
        tc.tile_pool(name="ps_s", bufs=1, space="PSUM") as ps_s,
    ):
        # ---- constants ----
        ident = cons.tile([P, P], F32)
        masks.make_identity(nc, ident[:])
        ones_col = cons.tile([P, 1], F32)
        nc.vector.memset(ones_col[:], 1.0)
        quarter_col = cons.tile([P, 1], F32)
        nc.vector.memset(quarter_col[:], 0.25)
        ones_row = cons.tile([1, P], F32)
        nc.vector.memset(ones_row[:], 1.0)
        magic = cons.tile([P, 2 * NB], U32)
        nc.vector.memset(magic[:], RSQRT_MAGIC)

        # cross-tile accumulators
        sel = cons.tile([P, 2 * NB], F32)    # cols 0..NB-1: s_plus, NB..: s_minus
        w2_all = cons.tile([P, NB], F32)     # ||w||^2 per tile column

        # PE warm-up on the identity: walrus allows only ONE sync wait per
        # matmul/LDW instruction, and the first real transpose would need
        # two (identity ready + r DMA done). This dummy consumes the
        # identity dependency so later PE instructions carry a single wait.
        warm = ps_t.tile([1, P], F32, tag="ptr")
        nc.tensor.transpose(warm[:1, :], ident[:, 0:1], ident[:])

        # ---- r_embedding prep: rT[k] = -2 * R^T chunk, r2_row = ||r||^2 ----
        r_sb = rp.tile([R, D], F32)
        nc.sync.dma_start(out=r_sb[:], in_=r[:, :])
        rT = []
        rsqs = []
        for k in range(ND):
            pt = ps_t.tile([P, R], F32, tag="ptr")
            nc.tensor.transpose(pt[:], r_sb[:, k * P:(k + 1) * P], ident[:R, :R])
            rt_k = rp.tile([P, R], F32, tag=f"rt{k}")
            nc.scalar.mul(rt_k[:], pt[:], -2.0)
            rT.append(rt_k)
            # (-2r)^2 = 4*r^2 exactly; the 0.25 weight below compensates.
            rsq_k = rp.tile([P, R], F32, tag=f"rsq{k}")
            nc.vector.tensor_tensor(rsq_k[:], rt_k[:], rt_k[:], op=OP.mult)
            rsqs.append(rsq_k)
        psum_r2 = ps_s.tile([1, R], F32, tag="acc")
        for k in range(ND):
            nc.tensor.matmul(
                psum_r2[:], quarter_col[:], rsqs[k][:],
                start=(k == 0), stop=(k == ND - 1))
        r2_row = rp.tile([1, R], F32)
        nc.vector.tensor_copy(r2_row[:], psum_r2[:])

        # ---- main loop over batch tiles ----
        for i in range(NB):
            w_sb = wload.tile([P, D], F32, tag="w")
            nc.sync.dma_start(out=w_sb[:], in_=w[i * P:(i + 1) * P, :])
            lab_sb = sm.tile([P, R], F32, tag="lab")
            nc.sync.dma_start(out=lab_sb[:], in_=lab[i * P:(i + 1) * P, :])

            # ||w||^2 per row (only shifts dist^2 uniformly; ordering unaffected)
            wsq = wload.tile([P, D], F32, tag="wsq")
            nc.vector.tensor_tensor(wsq[:], w_sb[:], w_sb[:], op=OP.mult)
            nc.vector.reduce_sum(w2_all[:, i:i + 1], wsq[:], axis=AX.X)

            # W^T chunks via PE transpose (psum -> sbuf copy split ACT/DVE)
            wts = []
            for k in range(ND):
                ptw = ps_t.tile([P, P], F32, tag="ptw")
                nc.tensor.transpose(ptw[:], w_sb[:, k * P:(k + 1) * P], ident[:])
                wt_sb = wtp.tile([P, P], F32, tag=f"wt{k}")
                nc.scalar.copy(wt_sb[:], ptw[:])
                wts.append(wt_sb)

            # s = -2 W R^T + r2  (single PSUM accumulation group)
            ps = ps_d.tile([P, R], F32, tag="s")
            for k in range(ND):
                nc.tensor.matmul(
                    ps[:], wts[k][:], rT[k][:], start=(k == 0), stop=False)
            nc.tensor.matmul(
                ps[:], ones_row[:1, :], r2_row[:1, :], start=False, stop=True)

            # row max and one-hot pred
            m1 = sm.tile([P, 1], F32, tag="m1")
            nc.vector.reduce_max(m1[:], ps[:], axis=AX.X)
            pred_sb = sm.tile([P, R], F32, tag="pred")
            nc.vector.tensor_scalar(
                out=pred_sb[:], in0=ps[:], scalar1=m1[:], scalar2=None,
                op0=OP.is_equal)
            nc.sync.dma_start(out=pred[i * P:(i + 1) * P, :], in_=pred_sb[:])

            # label one-hot
            lm = sm.tile([P, 1], F32, tag="lm")
            nc.vector.reduce_max(lm[:], lab_sb[:], axis=AX.X)
            oh = sm.tile([P, R], F32, tag="oh")
            nc.vector.tensor_scalar(
                out=oh[:], in0=lab_sb[:], scalar1=lm[:], scalar2=None,
                op0=OP.is_equal)

            # s_plus = s[b, y]  (gather via one-hot row-sum)
            scr = sm.tile([P, R], F32, tag="scr")
            nc.vector.scalar_tensor_tensor(
                out=scr[:], in0=ps[:], scalar=1.0, in1=oh[:],
                op0=OP.mult, op1=OP.mult, accum_out=sel[:, i:i + 1])

            # t = (argmax == y)  as 1.0/0.0
            t_col = sm.tile([P, 1], F32, tag="t")
            nc.vector.tensor_scalar(
                out=t_col[:], in0=sel[:, i:i + 1], scalar1=m1[:], scalar2=None,
                op0=OP.is_equal)

            # dx = (pred - 1) * s  -> row min = -(2nd max of s); s > 0 here.
            dx = sm.tile([P, R], F32, tag="dx")
            nc.vector.scalar_tensor_tensor(
                out=dx[:], in0=pred_sb[:], scalar=1.0, in1=ps[:],
                op0=OP.subtract, op1=OP.mult)
            m2n = sm.tile([P, 1], F32, tag="m2n")
            nc.vector.tensor_reduce(m2n[:], dx[:], axis=AX.X, op=OP.min)

            # s_minus = m1 + t * (m2 - m1);  m2 = -m2n
            mdiff = sm.tile([P, 1], F32, tag="mdiff")
            nc.vector.scalar_tensor_tensor(
                out=mdiff[:], in0=m2n[:], scalar=-1.0, in1=m1[:],
                op0=OP.mult, op1=OP.subtract)
            nc.vector.scalar_tensor_tensor(
                out=sel[:, NB + i:NB + i + 1], in0=mdiff[:], scalar=t_col[:],
                in1=m1[:], op0=OP.mult, op1=OP.add)

        # ---- epilogue: dist = sqrt(w2 + s_sel) for the 2*NB selected cols ----
        xs = cons.tile([P, 2 * NB], F32)
        nc.vector.tensor_tensor(xs[:, 0:NB], sel[:, 0:NB], w2_all[:], op=OP.add)
        nc.vector.tensor_tensor(
            xs[:, NB:2 * NB], sel[:, NB:2 * NB], w2_all[:], op=OP.add)

        # y0 = bit-trick rsqrt seed
        y = cons.tile([P, 2 * NB], U32)
        nc.vector.tensor_scalar(
            out=y[:], in0=xs[:].bitcast(U32), scalar1=1, scalar2=None,
            op0=OP.logical_shift_right)
        nc.vector.tensor_tensor(y[:], magic[:], y[:], op=OP.subtract)
        yf = y[:].bitcast(F32)
        # 3 Newton iterations: y <- y * (1.5 - 0.5 * x * y^2)
        y2 = cons.tile([P, 2 * NB], F32)
        a = cons.tile([P, 2 * NB], F32)
        for _ in range(3):
            nc.vector.tensor_tensor(y2[:], yf, yf, op=OP.mult)
            nc.vector.tensor_tensor(a[:], xs[:], y2[:], op=OP.mult)
            nc.vector.tensor_scalar(
                out=a[:], in0=a[:], scalar1=-0.5, scalar2=1.5,
                op0=OP.mult, op1=OP.add)
            nc.vector.tensor_tensor(yf, yf, a[:], op=OP.mult)
        dist_sel = cons.tile([P, 2 * NB], F32)
        nc.vector.tensor_tensor(dist_sel[:], xs[:], yf, op=OP.mult)

        # per-row plus - minus, then partition sum via PE
        pm = cons.tile([P, NB], F32)
        nc.vector.tensor_tensor(
            pm[:], dist_sel[:, 0:NB], dist_sel[:, NB:2 * NB], op=OP.subtract)
        pm_sum = cons.tile([P, 1], F32)
        nc.vector.reduce_sum(pm_sum[:], pm[:], axis=AX.X)
        psum_l = ps_s.tile([1, 1], F32, tag="acc")
        nc.tensor.matmul(psum_l[:], pm_sum[:], ones_col[:], start=True, stop=True)
        loss_sb = cons.tile([1, 1], F32)
        nc.vector.tensor_copy(loss_sb[:], psum_l[:])
        nc.sync.dma_start(out=lsum[:, :], in_=loss_sb[:])

    return nc


_NC_CACHE = []


def _get_nc():
    if not _NC_CACHE:
        _NC_CACHE.append(build_bass())
    return _NC_CACHE[0]


def execute(in_maps, **kwargs):
    nc = _get_nc()
    return run_bass_kernel_spmd(nc, in_maps, core_ids=list(range(N_CORES)), **kwargs)


def make_in_maps(w_output, r_embedding, label):
    w_output = np.ascontiguousarray(np.asarray(w_output, dtype=np.float32))
    r_embedding = np.ascontiguousarray(np.asarray(r_embedding, dtype=np.float32))
    label = np.ascontiguousarray(np.asarray(label, dtype=np.float32))
    in_maps = []
    for c in range(N_CORES):
        sl = slice(c * BS, (c + 1) * BS)
        in_maps.append({"w": w_output[sl], "r": r_embedding, "lab": label[sl]})
    return in_maps


def assemble(results):
    pred = np.concatenate([res["pred"] for res in results], axis=0)
    total = float(np.sum([np.float64(res["lsum"][0, 0]) for res in results]))
    loss = np.float32(1.0 + total / B)
    return pred, np.array(loss, dtype=np.float32)


def kernel(w_output, r_embedding, label):
    res = execute(make_in_maps(w_output, r_embedding, label))
    return assemble(res.results)


# revision 7
# speedup vs baseline: 1.0141x; 1.0141x over previous
"""Trainium2 Bass kernel for nn_LossLayer (distance loss_fn).

reference semantics:
    dist[b, r] = || w[b] - r_emb[r] ||_2         (B=4096, R=64, D=512)
    pred = one_hot(argmax_r dist)                [B, R] f32
    y = argmax_r label
    plus = dist[b, y]
    minus = dist[b, top1] if top1 != y else dist[b, top2]
    loss = mean(1 + plus - minus)                scalar f32

Strategy (data-parallel over batch on 8 cores, r_embedding replicated):
    s[b, r] = ||r_r||^2 - 2 w_b . r_r   (= dist^2 - ||w_b||^2, same per-row order)
    All argmax / top-2 / gather selections are done on s (exact f32 values from
    PSUM; sqrt is monotonic so ordering matches dist). sqrt is needed only for
    the 2 selected values per row (plus/minus); computed with the rsqrt
    bit-trick + 3 Newton iterations on the vector engine (f32-accurate,
    avoiding the low-precision ACT sqrt table).
    Each core emits its pred shard [512, 64] and a partial sum of
    (plus - minus) over its rows; host combines: loss = 1 + total/B.
"""

import sys

import numpy as np

try:
    import concourse.bass as bass
except ImportError:  # pragma: no cover
    sys.path.insert(0, "/opt/trn_rl_repo")
    import concourse.bass as bass

import json

import concourse.mybir as mybir
from concourse import bass2jax, bass_utils, masks
from concourse.bass_utils import run_bass_kernel_spmd
from concourse.tile import TileContext

# ---------------------------------------------------------------------------
# This container's walrus build rejects (a) the EVENT_SEMAPHORE_RANGE_CLEAR
# encoding Tile emits at kernel end ("ISA wrong length") and (b) instructions
# carrying more than one sync wait ("Too many sync wait commands").  Rewrite
# the BIR JSON just before walrus: drop the range-clear, and hoist extra
# waits onto standalone single-wait EventSemaphore instructions inserted
# immediately before (same engine => per-engine order preserved).
# ---------------------------------------------------------------------------


def _rewrite_bir(j):
    for fn in j.get("functions", []):
        for blk in fn.get("blocks", []):
            out = []
            for inst in blk.get("instructions", []):
                if inst.get("isa_opcode") == 176:  # EVSEM RANGE_CLEAR
                    continue
                si = inst.get("sync_info")
                waits = (si or {}).get("on_wait") or []
                if len(waits) > 1:
                    for idx, wv in enumerate(waits[:-1]):
                        out.append({
                            "debug": inst.get("debug", 0),
                            "engine": inst["engine"],
                            "ins": [],
                            "outs": [],
                            "name": f"{inst['name']}-wsplit{idx}",
                            "opcode": "EventSemaphore",
                            "sync_info": {"on_update": [], "on_wait": [wv]},
                        })
                    si["on_wait"] = [waits[-1]]
                out.append(inst)
            blk["instructions"] = out
    return j


_ORIG_COMPILE = bass_utils.compile_bir_kernel


def _patched_compile_bir_kernel(bir_json, *args, **kwargs):
    if isinstance(bir_json, (bytes, bytearray)):
        j = json.loads(bir_json)
    else:
        j = json.loads(bir_json)
    bir_json = json.dumps(_rewrite_bir(j)).encode()
    return _ORIG_COMPILE(bir_json, *args, **kwargs)


if getattr(bass_utils.compile_bir_kernel, "__name__", "") != "_patched_compile_bir_kernel":
    bass_utils.compile_bir_kernel = _patched_compile_bir_kernel
    bass2jax.compile_bir_kernel = _patched_compile_bir_kernel

F32 = mybir.dt.float32
U32 = mybir.dt.uint32
OP = mybir.AluOpType
AX = mybir.AxisListType

B, R, D = 4096, 64, 512
N_CORES = 8
BS = B // N_CORES          # 512 rows per core
P = 128                    # partitions
NB = BS // P               # 4 batch tiles per core
ND = D // P                # 4 contraction chunks

RSQRT_MAGIC = 0x5F3759DF


def build_bass():
    nc = bass.Bass()
    w = nc.dram_tensor("w", [BS, D], F32, kind="ExternalInput")
    r = nc.dram_tensor("r", [R, D], F32, kind="ExternalInput")
    lab = nc.dram_tensor("lab", [BS, R], F32, kind="ExternalInput")
    pred = nc.dram_tensor("pred", [BS, R], F32, kind="ExternalOutput")
    lsum = nc.dram_tensor("lsum", [1, 1], F32, kind="ExternalOutput")

    with (
        TileContext(nc) as tc,
        tc.tile_pool(name="const", bufs=1) as cons,
        tc.tile_pool(name="rprep", bufs=1) as rp,
        tc.tile_pool(name="wload", bufs=3) as wload,
        tc.tile_pool(name="wt", bufs=2) as wtp,
        tc.tile_pool(name="small", bufs=2) as sm,
        tc.tile_pool(name="ps_t", bufs=2, space="PSUM") as ps_t,
        tc.tile_pool(name="ps_d", bufs=2, space="PSUM") as ps_d,
        tc.tile_pool(name="ps_s", bufs=1, space="PSUM") as ps_s,
    ):
        # ---- constants ----
        ident = cons.tile([P, P], F32)
        masks.make_identity(nc, ident[:])
        ones_col = cons.tile([P, 1], F32)
        nc.vector.memset(ones_col[:], 1.0)
        quarter_col = cons.tile([P, 1], F32)
        nc.vector.memset(quarter_col[:], 0.25)
        ones_row = cons.tile([1, P], F32)
        nc.vector.memset(ones_row[:], 1.0)
        magic = cons.tile([P, 2 * NB], U32)
        nc.vector.memset(magic[:], RSQRT_MAGIC)

        # cross-tile accumulators
        sel = cons.tile([P, 2 * NB], F32)    # cols 0..NB-1: s_plus, NB..: s_minus
        w2_all = cons.tile([P, NB], F32)     # ||w||^2 per tile column

        # PE warm-up on the identity: walrus allows only ONE sync wait per
        # matmul/LDW instruction, and the first real transpose would need
        # two (identity ready + r DMA done). This dummy consumes the
        # identity dependency so later PE instructions carry a single wait.
        warm = ps_t.tile([1, P], F32, tag="ptr")
        nc.tensor.transpose(warm[:1, :], ident[:, 0:1], ident[:])

        # ---- r_embedding prep: rT[k] = -2 * R^T chunk, r2_row = ||r||^2 ----
        r_sb = rp.tile([R, D], F32)
        nc.sync.dma_start(out=r_sb[:], in_=r[:, :])
        rT = []
        rsqs = []
        for k in range(ND):
            pt = ps_t.tile([P, R], F32, tag="ptr")
            nc.tensor.transpose(pt[:], r_sb[:, k * P:(k + 1) * P], ident[:R, :R])
            rt_k = rp.tile([P, R], F32, tag=f"rt{k}")
            nc.scalar.mul(rt_k[:], pt[:], -2.0)
            rT.append(rt_k)
            # (-2r)^2 = 4*r^2 exactly; the 0.25 weight below compensates.
            rsq_k = rp.tile([P, R], F32, tag=f"rsq{k}")
            nc.vector.tensor_tensor(rsq_k[:], rt_k[:], rt_k[:], op=OP.mult)
            rsqs.append(rsq_k)
        psum_r2 = ps_s.tile([1, R], F32, tag="acc")
        for k in range(ND):
            nc.tensor.matmul(
                psum_r2[:], quarter_col[:], rsqs[k][:],
                start=(k == 0), stop=(k == ND - 1))
        r2_row = rp.tile([1, R], F32)
        nc.vector.tensor_copy(r2_row[:], psum_r2[:])

        # ---- main loop over batch tiles ----
        for i in range(NB):
            w_sb = wload.tile([P, D], F32, tag="w")
            nc.sync.dma_start(out=w_sb[:], in_=w[i * P:(i + 1) * P, :])
            lab_sb = sm.tile([P, R], F32, tag="lab")
            nc.sync.dma_start(out=lab_sb[:], in_=lab[i * P:(i + 1) * P, :])

            # ||w||^2 per row (only shifts dist^2 uniformly; ordering unaffected)
            wsq = wload.tile([P, D], F32, tag="wsq")
            nc.vector.tensor_tensor(wsq[:], w_sb[:], w_sb[:], op=OP.mult)
            nc.vector.reduce_sum(w2_all[:, i:i + 1], wsq[:], axis=AX.X)

            # W^T chunks via PE transpose (psum -> sbuf copy split ACT/DVE)
            wts = []
            for k in range(ND):
                ptw = ps_t.tile([P, P], F32, tag="ptw")
                nc.tensor.transpose(ptw[:], w_sb[:, k * P:(k + 1) * P], ident[:])
                wt_sb = wtp.tile([P, P], F32, tag=f"wt{k}")
                nc.scalar.copy(wt_sb[:], ptw[:])
                wts.append(wt_sb)

            # s = -2 W R^T + r2  (single PSUM accumulation group)
            ps = ps_d.tile([P, R], F32, tag="s")
            for k in range(ND):
                nc.tensor.matmul(
                    ps[:], wts[k][:], rT[k][:], start=(k == 0), stop=False)
            nc.tensor.matmul(
                ps[:], ones_row[:1, :], r2_row[:1, :], start=False, stop=True)

            # row max and one-hot pred
            m1 = sm.tile([P, 1], F32, tag="m1")
            nc.vector.reduce_max(m1[:], ps[:], axis=AX.X)
            pred_sb = sm.tile([P, R], F32, tag="pred")
            nc.vector.tensor_scalar(
                out=pred_sb[:], in0=ps[:], scalar1=m1[:], scalar2=None,
                op0=OP.is_equal)
            nc.sync.dma_start(out=pred[i * P:(i + 1) * P, :], in_=pred_sb[:])

            # label one-hot
            lm = sm.tile([P, 1], F32, tag="lm")
            nc.vector.reduce_max(lm[:], lab_sb[:], axis=AX.X)
            oh = sm.tile([P, R], F32, tag="oh")
            nc.vector.tensor_scalar(
                out=oh[:], in0=lab_sb[:], scalar1=lm[:], scalar2=None,
                op0=OP.is_equal)

            # s_plus = s[b, y]  (gather via one-hot row-sum)
            scr = sm.tile([P, R], F32, tag="scr")
            nc.vector.scalar_tensor_tensor(
                out=scr[:], in0=ps[:], scalar=1.0, in1=oh[:],
                op0=OP.mult, op1=OP.mult, accum_out=sel[:, i:i + 1])

            # t = (argmax == y)  as 1.0/0.0
            t_col = sm.tile([P, 1], F32, tag="t")
            nc.vector.tensor_scalar(
                out=t_col[:], in0=sel[:, i:i + 1], scalar1=m1[:], scalar2=None,
                op0=OP.is_equal)

            # dx = (pred - 1) * s  -> row min = -(2nd max of s); s > 0 here.
            dx = sm.tile([P, R], F32, tag="dx")
            nc.vector.scalar_tensor_tensor(
                out=dx[:], in0=pred_sb[:], scalar=1.0, in1=ps[:],
                op0=OP.subtract, op1=OP.mult)
            m2n = sm.tile([P, 1], F32, tag="m2n")
            nc.vector.tensor_reduce(m2n[:], dx[:], axis=AX.X, op=OP.min)

            # s_minus = m1 + t * (m2 - m1);  m2 = -m2n
            mdiff = sm.tile([P, 1], F32, tag="mdiff")
            nc.vector.scalar_tensor_tensor(
                out=mdiff[:], in0=m2n[:], scalar=-1.0, in1=m1[:],
                op0=OP.mult, op1=OP.subtract)
            nc.vector.scalar_tensor_tensor(
                out=sel[:, NB + i:NB + i + 1], in0=mdiff[:], scalar=t_col[:],
                in1=m1[:], op0=OP.mult, op1=OP.add)

        # ---- epilogue: dist = sqrt(w2 + s_sel) for the 2*NB selected cols ----
        xs = cons.tile([P, 2 * NB], F32)
        nc.vector.tensor_tensor(xs[:, 0:NB], sel[:, 0:NB], w2_all[:], op=OP.add)
        nc.vector.tensor_tensor(
            xs[:, NB:2 * NB], sel[:, NB:2 * NB], w2_all[:], op=OP.add)

        # y0 = bit-trick rsqrt seed
        y = cons.tile([P, 2 * NB], U32)
        nc.vector.tensor_scalar(
            out=y[:], in0=xs[:].bitcast(U32), scalar1=1, scalar2=None,
            op0=OP.logical_shift_right)
        nc.vector.tensor_tensor(y[:], magic[:], y[:], op=OP.subtract)
        yf = y[:].bitcast(F32)
        # 3 Newton iterations: y <- y * (1.5 - 0.5 * x * y^2)
        y2 = cons.tile([P, 2 * NB], F32)
        a = cons.tile([P, 2 * NB], F32)
        for _ in range(3):
            nc.vector.tensor_tensor(y2[:], yf, yf, op=OP.mult)
            nc.vector.tensor_tensor(a[:], xs[:], y2[:], op=OP.mult)
            nc.vector.tensor_scalar(
                out=a[:], in0=a[:], scalar1=-0.5, scalar2=1.5,
                op0=OP.mult, op1=OP.add)
            nc.vector.tensor_tensor(yf, yf, a[:], op=OP.mult)
        dist_sel = cons.tile([P, 2 * NB], F32)
        nc.vector.tensor_tensor(dist_sel[:], xs[:], yf, op=OP.mult)

        # per-row plus - minus, then partition sum via PE
        pm = cons.tile([P, NB], F32)
        nc.vector.tensor_tensor(
            pm[:], dist_sel[:, 0:NB], dist_sel[:, NB:2 * NB], op=OP.subtract)
        pm_sum = cons.tile([P, 1], F32)
        nc.vector.reduce_sum(pm_sum[:], pm[:], axis=AX.X)
        psum_l = ps_s.tile([1, 1], F32, tag="acc")
        nc.tensor.matmul(psum_l[:], pm_sum[:], ones_col[:], start=True, stop=True)
        loss_sb = cons.tile([1, 1], F32)
        nc.vector.tensor_copy(loss_sb[:], psum_l[:])
        nc.sync.dma_start(out=lsum[:, :], in_=loss_sb[:])

    return nc


_NC_CACHE = []


def _get_nc():
    if not _NC_CACHE:
        _NC_CACHE.append(build_bass())
    return _NC_CACHE[0]


def execute(in_maps, **kwargs):
    nc = _get_nc()
    return run_bass_kernel_spmd(nc, in_maps, core_ids=list(range(N_CORES)), **kwargs)


def make_in_maps(w_output, r_embedding, label):
    w_output = np.ascontiguousarray(np.asarray(w_output, dtype=np.float32))
    r_embedding = np.ascontiguousarray(np.asarray(r_embedding, dtype=np.float32))
    label = np.ascontiguousarray(np.asarray(label, dtype=np.float32))
    in_maps = []
    for c in range(N_CORES):
        sl = slice(c * BS, (c + 1) * BS)
        in_maps.append({"w": w_output[sl], "r": r_embedding, "lab": label[sl]})
    return in_maps


def assemble(results):
    pred = np.concatenate([res["pred"] for res in results], axis=0)
    total = float(np.sum([np.float64(res["lsum"][0, 0]) for res in results]))
    loss = np.float32(1.0 + total / B)
    return pred, np.array(loss, dtype=np.float32)


def kernel(w_output, r_embedding, label):
    res = execute(make_in_maps(w_output, r_embedding, label))
    return assemble(res.results)


# revision 9
# speedup vs baseline: 1.1258x; 1.1101x over previous
"""Trainium2 Bass kernel for nn_LossLayer (distance loss_fn).

reference semantics:
    dist[b, r] = || w[b] - r_emb[r] ||_2         (B=4096, R=64, D=512)
    pred = one_hot(argmax_r dist)                [B, R] f32
    y = argmax_r label
    plus = dist[b, y]
    minus = dist[b, top1] if top1 != y else dist[b, top2]
    loss = mean(1 + plus - minus)                scalar f32

Strategy (data-parallel over batch on 8 cores, r_embedding replicated):
    s[b, r] = ||r_r||^2 - 2 w_b . r_r   (= dist^2 - ||w_b||^2, same per-row order)
    All argmax / top-2 / gather selections are done on s (exact f32 values from
    PSUM; sqrt is monotonic so ordering matches dist). sqrt is needed only for
    the 2 selected values per row (plus/minus); computed with the rsqrt
    bit-trick + 3 Newton iterations on the vector engine (f32-accurate,
    avoiding the low-precision ACT sqrt table).
    Each core emits its pred shard [512, 64] and a partial sum of
    (plus - minus) over its rows; host combines: loss = 1 + total/B.
"""

import sys

import numpy as np

try:
    import concourse.bass as bass
except ImportError:  # pragma: no cover
    sys.path.insert(0, "/opt/trn_rl_repo")
    import concourse.bass as bass

import json

import concourse.mybir as mybir
from concourse import bass2jax, bass_utils, masks
from concourse.bass_utils import run_bass_kernel_spmd
from concourse.tile import TileContext

# ---------------------------------------------------------------------------
# This container's walrus build rejects (a) the EVENT_SEMAPHORE_RANGE_CLEAR
# encoding Tile emits at kernel end ("ISA wrong length") and (b) instructions
# carrying more than one sync wait ("Too many sync wait commands").  Rewrite
# the BIR JSON just before walrus: drop the range-clear, and hoist extra
# waits onto standalone single-wait EventSemaphore instructions inserted
# immediately before (same engine => per-engine order preserved).
# ---------------------------------------------------------------------------


def _rewrite_bir(j):
    for fn in j.get("functions", []):
        for blk in fn.get("blocks", []):
            out = []
            for inst in blk.get("instructions", []):
                if inst.get("isa_opcode") == 176:  # EVSEM RANGE_CLEAR
                    continue
                si = inst.get("sync_info")
                waits = (si or {}).get("on_wait") or []
                if len(waits) > 1:
                    for idx, wv in enumerate(waits[:-1]):
                        out.append({
                            "debug": inst.get("debug", 0),
                            "engine": inst["engine"],
                            "ins": [],
                            "outs": [],
                            "name": f"{inst['name']}-wsplit{idx}",
                            "opcode": "EventSemaphore",
                            "sync_info": {"on_update": [], "on_wait": [wv]},
                        })
                    si["on_wait"] = [waits[-1]]
                out.append(inst)
            blk["instructions"] = out
    return j


_ORIG_COMPILE = bass_utils.compile_bir_kernel


def _patched_compile_bir_kernel(bir_json, *args, **kwargs):
    if isinstance(bir_json, (bytes, bytearray)):
        j = json.loads(bir_json)
    else:
        j = json.loads(bir_json)
    bir_json = json.dumps(_rewrite_bir(j)).encode()
    return _ORIG_COMPILE(bir_json, *args, **kwargs)


if getattr(bass_utils.compile_bir_kernel, "__name__", "") != "_patched_compile_bir_kernel":
    bass_utils.compile_bir_kernel = _patched_compile_bir_kernel
    bass2jax.compile_bir_kernel = _patched_compile_bir_kernel

F32 = mybir.dt.float32
U32 = mybir.dt.uint32
OP = mybir.AluOpType
AX = mybir.AxisListType

B, R, D = 4096, 64, 512
N_CORES = 8
BS = B // N_CORES          # 512 rows per core
P = 128                    # partitions
NB = BS // P               # 4 batch tiles per core
ND = D // P                # 4 contraction chunks

RSQRT_MAGIC = 0x5F3759DF


def build_bass():
    nc = bass.Bass()
    w = nc.dram_tensor("w", [BS, D], F32, kind="ExternalInput")
    r = nc.dram_tensor("r", [R, D], F32, kind="ExternalInput")
    lab = nc.dram_tensor("lab", [BS, R], F32, kind="ExternalInput")
    pred = nc.dram_tensor("pred", [BS, R], F32, kind="ExternalOutput")
    lsum = nc.dram_tensor("lsum", [1, 1], F32, kind="ExternalOutput")

    with (
        TileContext(nc) as tc,
        tc.tile_pool(name="const", bufs=1) as cons,
        tc.tile_pool(name="rprep", bufs=1) as rp,
        tc.tile_pool(name="wload", bufs=3) as wload,
        tc.tile_pool(name="wt", bufs=2) as wtp,
        tc.tile_pool(name="small", bufs=2) as sm,
        tc.tile_pool(name="ps_t", bufs=2, space="PSUM") as ps_t,
        tc.tile_pool(name="ps_d", bufs=2, space="PSUM") as ps_d,
        tc.tile_pool(name="ps_s", bufs=1, space="PSUM") as ps_s,
    ):
        # ---- constants ----
        ident = cons.tile([P, P], F32)
        masks.make_identity(nc, ident[:])
        ones_col = cons.tile([P, 1], F32)
        nc.vector.memset(ones_col[:], 1.0)
        quarter_col = cons.tile([P, 1], F32)
        nc.vector.memset(quarter_col[:], 0.25)
        ones_row = cons.tile([1, P], F32)
        nc.vector.memset(ones_row[:], 1.0)
        magic = cons.tile([P, 2 * NB], U32)
        nc.vector.memset(magic[:], RSQRT_MAGIC)

        # cross-tile accumulators
        sel = cons.tile([P, 2 * NB], F32)    # cols 0..NB-1: s_plus, NB..: s_minus
        w2_all = cons.tile([P, NB], F32)     # ||w||^2 per tile column

        # PE warm-up on the identity: walrus allows only ONE sync wait per
        # matmul/LDW instruction, and the first real transpose would need
        # two (identity ready + r DMA done). This dummy consumes the
        # identity dependency so later PE instructions carry a single wait.
        warm = ps_t.tile([1, P], F32, tag="ptr")
        nc.tensor.transpose(warm[:1, :], ident[:, 0:1], ident[:])

        # ---- r_embedding prep: rT[k] = -2 * R^T chunk, r2_row = ||r||^2 ----
        r_sb = rp.tile([R, D], F32)
        nc.sync.dma_start(out=r_sb[:], in_=r[:, :])
        rT = []
        rsqs = []
        for k in range(ND):
            pt = ps_t.tile([P, R], F32, tag="ptr")
            nc.tensor.transpose(pt[:], r_sb[:, k * P:(k + 1) * P], ident[:R, :R])
            rt_k = rp.tile([P, R], F32, tag=f"rt{k}")
            nc.scalar.mul(rt_k[:], pt[:], -2.0)
            rT.append(rt_k)
            # (-2r)^2 = 4*r^2 exactly; the 0.25 weight below compensates.
            rsq_k = rp.tile([P, R], F32, tag=f"rsq{k}")
            nc.vector.tensor_tensor(rsq_k[:], rt_k[:], rt_k[:], op=OP.mult)
            rsqs.append(rsq_k)
        psum_r2 = ps_s.tile([1, R], F32, tag="acc")
        for k in range(ND):
            nc.tensor.matmul(
                psum_r2[:], quarter_col[:], rsqs[k][:],
                start=(k == 0), stop=(k == ND - 1))
        r2_row = rp.tile([1, R], F32)
        nc.vector.tensor_copy(r2_row[:], psum_r2[:])

        # ---- main loop over batch tiles ----
        for i in range(NB):
            w_sb = wload.tile([P, D], F32, tag="w")
            nc.sync.dma_start(out=w_sb[:], in_=w[i * P:(i + 1) * P, :])
            lab_sb = sm.tile([P, R], F32, tag="lab")
            nc.sync.dma_start(out=lab_sb[:], in_=lab[i * P:(i + 1) * P, :])

            # ||w||^2 per row (only shifts dist^2 uniformly; ordering unaffected)
            wsq = wload.tile([P, D], F32, tag="wsq")
            # square on GPSIMD (otherwise idle) to unload the DVE bottleneck
            nc.gpsimd.tensor_tensor(wsq[:], w_sb[:], w_sb[:], op=OP.mult)
            nc.vector.reduce_sum(w2_all[:, i:i + 1], wsq[:], axis=AX.X)

            # W^T chunks via PE transpose (psum -> sbuf copy split ACT/DVE)
            wts = []
            for k in range(ND):
                ptw = ps_t.tile([P, P], F32, tag="ptw")
                nc.tensor.transpose(ptw[:], w_sb[:, k * P:(k + 1) * P], ident[:])
                wt_sb = wtp.tile([P, P], F32, tag=f"wt{k}")
                nc.scalar.copy(wt_sb[:], ptw[:])
                wts.append(wt_sb)

            # s = -2 W R^T + r2  (single PSUM accumulation group)
            ps = ps_d.tile([P, R], F32, tag="s")
            for k in range(ND):
                nc.tensor.matmul(
                    ps[:], wts[k][:], rT[k][:], start=(k == 0), stop=False)
            nc.tensor.matmul(
                ps[:], ones_row[:1, :], r2_row[:1, :], start=False, stop=True)

            # row max and one-hot pred
            m1 = sm.tile([P, 1], F32, tag="m1")
            nc.vector.reduce_max(m1[:], ps[:], axis=AX.X)
            pred_sb = sm.tile([P, R], F32, tag="pred")
            nc.vector.tensor_scalar(
                out=pred_sb[:], in0=ps[:], scalar1=m1[:], scalar2=None,
                op0=OP.is_equal)
            nc.sync.dma_start(out=pred[i * P:(i + 1) * P, :], in_=pred_sb[:])

            # label one-hot
            lm = sm.tile([P, 1], F32, tag="lm")
            nc.vector.reduce_max(lm[:], lab_sb[:], axis=AX.X)
            oh = sm.tile([P, R], F32, tag="oh")
            nc.gpsimd.tensor_scalar(
                out=oh[:], in0=lab_sb[:], scalar1=lm[:], scalar2=None,
                op0=OP.is_equal)

            # s_plus = s[b, y]  (gather via one-hot row-sum)
            scr = sm.tile([P, R], F32, tag="scr")
            nc.vector.scalar_tensor_tensor(
                out=scr[:], in0=ps[:], scalar=1.0, in1=oh[:],
                op0=OP.mult, op1=OP.mult, accum_out=sel[:, i:i + 1])

            # t = (argmax == y)  as 1.0/0.0
            t_col = sm.tile([P, 1], F32, tag="t")
            nc.vector.tensor_scalar(
                out=t_col[:], in0=sel[:, i:i + 1], scalar1=m1[:], scalar2=None,
                op0=OP.is_equal)

            # dx = (pred - 1) * s  -> row min = -(2nd max of s); s > 0 here.
            dx = sm.tile([P, R], F32, tag="dx")
            nc.vector.scalar_tensor_tensor(
                out=dx[:], in0=pred_sb[:], scalar=1.0, in1=ps[:],
                op0=OP.subtract, op1=OP.mult)
            m2n = sm.tile([P, 1], F32, tag="m2n")
            nc.vector.tensor_reduce(m2n[:], dx[:], axis=AX.X, op=OP.min)

            # s_minus = m1 + t * (m2 - m1);  m2 = -m2n
            mdiff = sm.tile([P, 1], F32, tag="mdiff")
            nc.vector.scalar_tensor_tensor(
                out=mdiff[:], in0=m2n[:], scalar=-1.0, in1=m1[:],
                op0=OP.mult, op1=OP.subtract)
            nc.vector.scalar_tensor_tensor(
                out=sel[:, NB + i:NB + i + 1], in0=mdiff[:], scalar=t_col[:],
                in1=m1[:], op0=OP.mult, op1=OP.add)

        # ---- epilogue: dist = sqrt(w2 + s_sel) for the 2*NB selected cols ----
        xs = cons.tile([P, 2 * NB], F32)
        nc.vector.tensor_tensor(xs[:, 0:NB], sel[:, 0:NB], w2_all[:], op=OP.add)
        nc.vector.tensor_tensor(
            xs[:, NB:2 * NB], sel[:, NB:2 * NB], w2_all[:], op=OP.add)

        # y0 = bit-trick rsqrt seed
        y = cons.tile([P, 2 * NB], U32)
        nc.vector.tensor_scalar(
            out=y[:], in0=xs[:].bitcast(U32), scalar1=1, scalar2=None,
            op0=OP.logical_shift_right)
        nc.vector.tensor_tensor(y[:], magic[:], y[:], op=OP.subtract)
        yf = y[:].bitcast(F32)
        # 3 Newton iterations: y <- y * (1.5 - 0.5 * x * y^2)
        y2 = cons.tile([P, 2 * NB], F32)
        a = cons.tile([P, 2 * NB], F32)
        for _ in range(3):
            nc.vector.tensor_tensor(y2[:], yf, yf, op=OP.mult)
            nc.vector.tensor_tensor(a[:], xs[:], y2[:], op=OP.mult)
            nc.vector.tensor_scalar(
                out=a[:], in0=a[:], scalar1=-0.5, scalar2=1.5,
                op0=OP.mult, op1=OP.add)
            nc.vector.tensor_tensor(yf, yf, a[:], op=OP.mult)
        dist_sel = cons.tile([P, 2 * NB], F32)
        nc.vector.tensor_tensor(dist_sel[:], xs[:], yf, op=OP.mult)

        # per-row plus - minus, then partition sum via PE
        pm = cons.tile([P, NB], F32)
        nc.vector.tensor_tensor(
            pm[:], dist_sel[:, 0:NB], dist_sel[:, NB:2 * NB], op=OP.subtract)
        pm_sum = cons.tile([P, 1], F32)
        nc.vector.reduce_sum(pm_sum[:], pm[:], axis=AX.X)
        psum_l = ps_s.tile([1, 1], F32, tag="acc")
        nc.tensor.matmul(psum_l[:], pm_sum[:], ones_col[:], start=True, stop=True)
        loss_sb = cons.tile([1, 1], F32)
        nc.vector.tensor_copy(loss_sb[:], psum_l[:])
        nc.sync.dma_start(out=lsum[:, :], in_=loss_sb[:])

    return nc


_NC_CACHE = []


def _get_nc():
    if not _NC_CACHE:
        _NC_CACHE.append(build_bass())
    return _NC_CACHE[0]


def execute(in_maps, **kwargs):
    nc = _get_nc()
    return run_bass_kernel_spmd(nc, in_maps, core_ids=list(range(N_CORES)), **kwargs)


def make_in_maps(w_output, r_embedding, label):
    w_output = np.ascontiguousarray(np.asarray(w_output, dtype=np.float32))
    r_embedding = np.ascontiguousarray(np.asarray(r_embedding, dtype=np.float32))
    label = np.ascontiguousarray(np.asarray(label, dtype=np.float32))
    in_maps = []
    for c in range(N_CORES):
        sl = slice(c * BS, (c + 1) * BS)
        in_maps.append({"w": w_output[sl], "r": r_embedding, "lab": label[sl]})
    return in_maps


def assemble(results):
    pred = np.concatenate([res["pred"] for res in results], axis=0)
    total = float(np.sum([np.float64(res["lsum"][0, 0]) for res in results]))
    loss = np.float32(1.0 + total / B)
    return pred, np.array(loss, dtype=np.float32)


def kernel(w_output, r_embedding, label):
    res = execute(make_in_maps(w_output, r_embedding, label))
    return assemble(res.results)
